# revision 1
# baseline (speedup 1.0000x reference)
"""Trainium2 Bass kernel for the spectral history-filter model (nn_DSC_23252952941334).

Math: all four reference terms are linear in y_hist with per-lag effective
weight matrices, so the whole module collapses to

    out[b, c] = sum_{j=0..63} sum_p  W_time[j][c, p] * y_hist[b, j+2, p]

where W_time[j] combines M_bar_0 / M_bar / M_0l / M_il with the small
spectral coefficient vectors (folded on host in float64 — ~5 MFLOP).

Device strategy (8 cores, data-parallel over batch):
  - host: fold weights, transpose each core's y shard to [k, b] layout
    (k = (lag j, p) on SBUF partitions; contraction dim must be the
    partition dim for the PE array)
  - device: out[c, b] = sum_k w[k, c] * y[k, b] as 64 accumulated
    128-contraction matmuls into 4 PSUM banks (b free dim 512 each)
  - host: gather per-core [c, b] outputs, transpose to (B, 128)
"""

import os
import numpy as np
from concurrent.futures import ThreadPoolExecutor

N_CORES = 8
B, L, P, MC = 16384, 66, 128, 128
H, M = 16, 32
NLAG = 64            # lags d=0..63 <-> y time indices 2..65
BS = B // N_CORES    # 2048 batch rows per core
NBT = 4              # psum b-tiles of 512
KJ = None            # k-chunks per DMA super-chunk; set per mode below

# Data dtype for y/w on device.  The harness gate is rel_err < 2e-2;
# measured accuracy on real data: f16 2.5e-4, f32r 1.25e-4, f32 2.6e-7.
# f16 halves HBM traffic (the binding roofline): ~100us vs ~192us per core.
#   "f16"  -> float16 tiles (host casts)
#   "f32r" -> fp32 data, float32r matmuls (1 cyc/row at free>=256)
#   "f32"  -> plain fp32 matmuls (4 cyc/row, PE-bound)
MODE = os.environ.get("KERNEL_MODE", "f16")
TRACE = False        # test.py can flip this to get a profile

_cached_nc = {}


def _fold_weights(M_bar_0, M_bar, M_0l, M_il, sigma_powered, phi,
                  lambda_powered, varphi):
    """Return w_dev (P, NLAG, MC) fp32 with w_dev[p, j, c] = W_time[j][c, p]."""
    f8 = np.float64
    M_bar_0 = M_bar_0.astype(f8); M_bar = M_bar.astype(f8)
    M_0l = M_0l.astype(f8); M_il = M_il.astype(f8)
    sig = sigma_powered.astype(f8); lam = lambda_powered.astype(f8)
    phi = phi.astype(f8); varphi = varphi.astype(f8)

    # W_lag[d] acts on Yr[:, d] = y[:, L-1-d]
    W = np.zeros((NLAG, MC, P), f8)
    W[0] = M_bar_0

    # term 2: sum_i lam[i] * varphi[j, i] * M_bar[i] on lag j+1
    coef2 = varphi @ np.diag(lam)                       # (M, H) -> [j, i]
    W[1:M + 1] += np.einsum('ji,icp->jcp', coef2, M_bar)

    # term 3: sum_l sigma_ext[l] * phi_ext[k, l] * M_0l[l] on lag k+1
    sigma_ext = np.concatenate([[1.0], sig])            # (H+1,)
    phi_ext = np.concatenate([np.ones((M, 1)), phi], 1)  # (M, H+1)
    coef3 = phi_ext @ np.diag(sigma_ext)                # (M, H+1) -> [k, l]
    W[1:M + 1] += np.einsum('kl,lcp->kcp', coef3, M_0l)

    # term 4: anti-diagonal fold of varphi[j,i] phi_ext[k,l] comb[l,i] M_il[i,l]
    comb = sigma_ext[:, None] * lam[None, :]            # (H+1, H) -> [l, i]
    corr = np.zeros((2 * M - 1, H + 1, H), f8)          # [d, l, i]
    for j in range(M):
        for k in range(M):
            corr[j + k] += phi_ext[k][:, None] * varphi[j][None, :]
    C4 = corr * comb[None]                              # (2M-1, H+1, H)
    W[1:2 * M] += np.einsum('dli,ilcp->dcp', C4, M_il)

    # reorder to ascending time index: W_time[j] = W_lag[63 - j]
    # and lay out for SBUF: w_dev[p, j, c]
    w_dev = np.ascontiguousarray(W[::-1].transpose(2, 0, 1)).astype(np.float32)
    return w_dev


def _transpose_shards(y, npdt):
    """y (B, L, P) fp32 -> list of per-core yt (NLAG, P, BS) npdt,
    yt[j, p, b] = y[core*BS + b, j + 2, p]."""
    src = y[:, 2:, :]                  # (B, 64, 128) strided view
    shards = [np.empty((NLAG, P, BS), npdt) for _ in range(N_CORES)]
    BB = 128

    def work(args):
        ci, b0 = args
        blk = np.ascontiguousarray(src[ci * BS + b0: ci * BS + b0 + BB])
        if npdt != np.float32:
            blk = blk.astype(npdt)
        shards[ci][:, :, b0:b0 + BB] = blk.transpose(1, 2, 0)

    jobs = [(ci, b0) for ci in range(N_CORES) for b0 in range(0, BS, BB)]
    with ThreadPoolExecutor(8) as ex:
        list(ex.map(work, jobs))
    return shards


def _mode_params(mode):
    from concourse import mybir
    if mode == "f16":
        return dict(npdt=np.float16, ddt=mybir.dt.float16,
                    rdt=mybir.dt.float16, kj=4, ybufs=8)
    if mode == "f32r":
        return dict(npdt=np.float32, ddt=mybir.dt.float32,
                    rdt=mybir.dt.float32r, kj=2, ybufs=8)
    if mode == "f32":
        return dict(npdt=np.float32, ddt=mybir.dt.float32,
                    rdt=mybir.dt.float32, kj=2, ybufs=8)
    raise ValueError(mode)


def _build_nc(mode):
    import concourse.bass as bass
    import concourse.tile as tile
    from concourse import mybir
    from concourse.bass import ts
    from contextlib import ExitStack

    mp = _mode_params(mode)
    DT = mybir.dt.float32
    DDT = mp["ddt"]
    RDT = mp["rdt"]
    KJ = mp["kj"]

    nc = bass.Bass()
    yt = nc.declare_dram_parameter("yt", [NLAG, P, BS], DDT, isOutput=False)
    w = nc.declare_dram_parameter("w", [P, NLAG, MC], DDT, isOutput=False)
    out = nc.declare_dram_parameter("out", [MC, BS], DT, isOutput=True)

    NSUP = NLAG // KJ  # DMA super-chunks

    with ExitStack() as ctx:
        tc = ctx.enter_context(tile.TileContext(nc))
        wpool = ctx.enter_context(tc.tile_pool(name="wp", bufs=1))
        ypool = ctx.enter_context(
            tc.tile_pool(name="yp", bufs=mp["ybufs"]))
        opool = ctx.enter_context(tc.tile_pool(name="op", bufs=1))
        pspool = ctx.enter_context(tc.tile_pool(name="ps", bufs=1, space="PSUM"))

        wtile = wpool.tile([P, NLAG, MC], RDT)
        nc.sync.dma_start(wtile[:], w[:].bitcast(RDT))

        psums = [pspool.tile([MC, 512], mybir.dt.float32, name=f"ps{t}")
                 for t in range(NBT)]

        # Warm-up matmuls consuming wtile: give the PE a single-wait
        # observation of the w-DMA (walrus rejects >1 sync wait on a
        # matmul) and ramp the HAM clock gate to 8/8 before the real
        # work (~2us of dense PE activity during the first y transfer).
        warm = pspool.tile([MC, 512], mybir.dt.float32, name="warm")
        for wi in range(8):
            nc.tensor.matmul(warm[:], wtile[:, wi, :],
                             wtile[:, 8 * wi:8 * wi + 4, :],
                             start=(wi == 0), stop=(wi == 7))

        for kk in range(NSUP):
            ytile = ypool.tile([P, KJ, BS], RDT)
            src = yt[kk * KJ:(kk + 1) * KJ, :, :].rearrange("j p b -> p j b")
            nc.sync.dma_start(ytile[:], src.bitcast(RDT))
            for jj in range(KJ):
                j = kk * KJ + jj
                lhsT = wtile[:, j, :]
                for t in range(NBT):
                    rhs = ytile[:, jj, ts(t, 512)]
                    nc.tensor.matmul(psums[t][:], lhsT, rhs,
                                     start=(j == 0), stop=(j == NLAG - 1))

        outt = opool.tile([MC, BS], DT)
        for t in range(NBT):
            nc.vector.tensor_copy(outt[:, ts(t, 512)], psums[t][:])
        nc.sync.dma_start(out[:], outt[:])

    return nc


def _strip_redundant_waits(nc):
    """Drop semaphore waits that are provably implied by other waits.

    Tile's add_semaphores pass is per-processor minimal but not transitively
    minimal; walrus codegen allows only one sync wait per DMA/Matmult/Drain
    HW instruction.  Model:
      - expand(s >= v) = {s >= v} union C[sat(s, v)] where sat is the
        instruction whose semaphore update first reaches v (updates on one
        engine sem / one DMA lane fire in order).
      - C[i] ("true once i's updates fired") = own updates + expand(own
        waits) + dispatch-knowledge (expand of same-engine predecessors'
        waits; sequencers evaluate waits in program order) + C[predecessor]
        chained in completion order: same engine for compute engines, same
        DMA lane for DMACopy (async transfers complete in ring order).
    A wait w on i is droppable iff w is in (expand of same-engine
    predecessors' waits) union (expand of i's other waits).
    """
    from concourse import mybir

    f = nc.m.functions[0]
    insts = [i for blk in f.blocks for i in blk.instructions]

    def waits(i):
        si = i.sync_info
        return [(w.ant_name, w.wait_value) for w in (si.on_wait or [])] \
            if si else []

    def updates(i):
        si = i.sync_info
        return list(si.on_update or []) if si else []

    by_engine = {}
    for i in insts:
        by_engine.setdefault(str(i.engine), []).append(i)

    COMPUTE = {"EngineType.PE", "EngineType.DVE", "EngineType.Activation",
               "EngineType.Pool"}

    # cumulative update values per sem, in program order of the updater
    sem_updates = {}           # sem -> [(inst_name, cumulative)]
    upd_of = {i.name: [] for i in insts}
    for eng, lst in by_engine.items():
        for i in lst:
            for u in updates(i):
                cum = sem_updates.setdefault(u.ant_name, [])
                prev = cum[-1][1] if cum else 0
                val = u.update_value if u.update_mode == "sem-add-imm" else 1
                cum.append((i.name, prev + val))
                upd_of[i.name].append((u.ant_name, prev + val))

    def satisfier(sem, v):
        for name, val in sem_updates.get(sem, ()):
            if val >= v:
                return name
        return None

    # completion-order predecessor: same engine (compute) or same DMA lane
    comp_pred = {}
    last_on_lane = {}
    for eng, lst in by_engine.items():
        prev = None
        for i in lst:
            if type(i).__name__ == "InstDMACopy":
                lanes = [s for s, _ in upd_of[i.name]]
                lane = lanes[0] if lanes else None
                comp_pred[i.name] = last_on_lane.get(lane)
                if lane is not None:
                    last_on_lane[lane] = i.name
            elif eng in COMPUTE:
                comp_pred[i.name] = prev
            else:
                comp_pred[i.name] = None
            prev = i.name

    # dispatch-order predecessor (same engine, any type)
    disp_pred = {}
    for eng, lst in by_engine.items():
        prev = None
        for i in lst:
            disp_pred[i.name] = prev
            prev = i.name

    C = {i.name: {} for i in insts}
    DW = {i.name: {} for i in insts}   # dispatch knowledge (pred waits, cum.)
    name2inst = {i.name: i for i in insts}

    def merge(dst, src_items):
        ch = False
        for s, v in src_items:
            if dst.get(s, 0) < v:
                dst[s] = v
                ch = True
        return ch

    changed = True
    rounds = 0
    while changed and rounds < 100:
        changed = False
        rounds += 1
        for i in insts:
            n = i.name
            # DW: dispatch knowledge = pred's DW + expand(pred's waits)
            dp = disp_pred[n]
            if dp is not None:
                changed |= merge(DW[n], DW[dp].items())
                for s, v in waits(name2inst[dp]):
                    changed |= merge(DW[n], [(s, v)])
                    j = satisfier(s, v)
                    if j is not None:
                        changed |= merge(DW[n], C[j].items())
            # C: completion closure
            changed |= merge(C[n], DW[n].items())
            changed |= merge(C[n], upd_of[n])
            cp = comp_pred.get(n)
            if cp is not None:
                changed |= merge(C[n], C[cp].items())
            for s, v in waits(i):
                changed |= merge(C[n], [(s, v)])
                j = satisfier(s, v)
                if j is not None:
                    changed |= merge(C[n], C[j].items())

    for i in insts:
        si = i.sync_info
        if not si or len(si.on_wait or []) <= 1:
            continue
        if type(i).__name__ not in ("InstDMACopy", "InstMatmult",
                                    "InstDrain"):
            continue
        keep = []
        for w in si.on_wait:
            avail = dict(DW[i.name])
            for w2 in si.on_wait:
                if w2 is w:
                    continue
                merge(avail, [(w2.ant_name, w2.wait_value)])
                j = satisfier(w2.ant_name, w2.wait_value)
                if j is not None:
                    merge(avail, C[j].items())
            if avail.get(w.ant_name, 0) < w.wait_value:
                keep.append(w)
        if len(keep) > 1:
            raise RuntimeError(
                f"{i.name}: still {len(keep)} waits after stripping: "
                f"{[(w.ant_name, w.wait_value) for w in keep]}")
        if len(keep) != len(si.on_wait):
            i.sync_info = mybir.SyncInfo(
                on_wait=keep, on_update=list(si.on_update or []))
    return nc


def _get_nc(mode):
    if mode not in _cached_nc:
        _cached_nc[mode] = _strip_redundant_waits(_build_nc(mode))
    return _cached_nc[mode]


def kernel(y_hist, M_bar_0, M_bar, M_0l, M_il, sigma_powered, phi,
           lambda_powered, varphi):
    from concourse.bass_utils import run_bass_kernel_spmd

    mp = _mode_params(MODE)
    y_hist = np.ascontiguousarray(np.asarray(y_hist, dtype=np.float32))
    w_dev = _fold_weights(np.asarray(M_bar_0), np.asarray(M_bar),
                          np.asarray(M_0l), np.asarray(M_il),
                          np.asarray(sigma_powered), np.asarray(phi),
                          np.asarray(lambda_powered), np.asarray(varphi))
    if mp["npdt"] != np.float32:
        w_dev = w_dev.astype(mp["npdt"])
    shards = _transpose_shards(y_hist, mp["npdt"])

    nc = _get_nc(MODE)
    in_maps = [{"yt": shards[ci], "w": w_dev} for ci in range(N_CORES)]
    res = run_bass_kernel_spmd(nc, in_maps, list(range(N_CORES)), trace=TRACE)

    if TRACE:
        kernel.last_result = res

    out = np.empty((B, MC), np.float32)
    for ci in range(N_CORES):
        out[ci * BS:(ci + 1) * BS] = res.results[ci]["out"].T
    return out



# revision 3
# speedup vs baseline: 1.5334x; 1.5334x over previous
"""Trainium2 Bass kernel for the spectral history-filter model (nn_DSC_23252952941334).

Math: all four reference terms are linear in y_hist with per-lag effective
weight matrices, so the whole module collapses to

    out[b, c] = sum_{j=0..63} sum_p  W_time[j][c, p] * y_hist[b, j+2, p]

where W_time[j] combines M_bar_0 / M_bar / M_0l / M_il with the small
spectral coefficient vectors (folded on host in float64 — ~5 MFLOP).

Device strategy (8 cores, data-parallel over batch):
  - host: fold weights, transpose each core's y shard to [p, j, b] layout
    (contraction dims (j, p) with p on SBUF partitions; p-major so each
    partition reads KJ*BS contiguous bytes per DMA chunk)
  - device: out[c, b] = sum_{j,p} w[p, j, c] * y[p, j, b] as 64 accumulated
    128-contraction matmuls into 4 PSUM banks (b free dim 512 each)
  - host: gather per-core [c, b] outputs, transpose to (B, 128)

The kernel is HBM-bound: per-core y traffic dominates.  The harness gate is
rel_err < 2e-2; measured end-to-end accuracy on the real data:
  y=f16  w=f16 : 2.5e-4   (33.6 MB y per core, ~128 us)
  y=e3m4 w=f16 : 1.15e-2  (16.8 MB y per core)  <- "f8" mode
Mixed-dtype matmul (f16 stationary x e3m4 moving) verified exact on HW
(FP22 datapath, e3m4 subnormals handled; probe_fp8.py).
"""

import os
import numpy as np
import ml_dtypes
from concurrent.futures import ThreadPoolExecutor

N_CORES = 8
B, L, P, MC = 16384, 66, 128, 128
H, M = 16, 32
NLAG = 64            # lags d=0..63 <-> y time indices 2..65
BS = B // N_CORES    # 2048 batch rows per core
NBT = 4              # psum b-tiles of 512

# MODE:
#   "f8"   -> y float8_e3m4 moving, w float16 stationary, out f16 (default)
#   "f16"  -> y/w float16
#   "f32r" -> fp32 data, float32r matmuls
#   "f32"  -> plain fp32 matmuls
MODE = os.environ.get("KERNEL_MODE", "f8")
TRACE = False        # test.py can flip this to get a profile

_cached_nc = {}


def _fold_weights(M_bar_0, M_bar, M_0l, M_il, sigma_powered, phi,
                  lambda_powered, varphi):
    """Return w_dev (P, NLAG, MC) fp32 with w_dev[p, j, c] = W_time[j][c, p]."""
    f8 = np.float64
    M_bar_0 = M_bar_0.astype(f8); M_bar = M_bar.astype(f8)
    M_0l = M_0l.astype(f8); M_il = M_il.astype(f8)
    sig = sigma_powered.astype(f8); lam = lambda_powered.astype(f8)
    phi = phi.astype(f8); varphi = varphi.astype(f8)

    # W_lag[d] acts on Yr[:, d] = y[:, L-1-d]
    W = np.zeros((NLAG, MC, P), f8)
    W[0] = M_bar_0

    # term 2: sum_i lam[i] * varphi[j, i] * M_bar[i] on lag j+1
    coef2 = varphi @ np.diag(lam)                       # (M, H) -> [j, i]
    W[1:M + 1] += np.einsum('ji,icp->jcp', coef2, M_bar)

    # term 3: sum_l sigma_ext[l] * phi_ext[k, l] * M_0l[l] on lag k+1
    sigma_ext = np.concatenate([[1.0], sig])            # (H+1,)
    phi_ext = np.concatenate([np.ones((M, 1)), phi], 1)  # (M, H+1)
    coef3 = phi_ext @ np.diag(sigma_ext)                # (M, H+1) -> [k, l]
    W[1:M + 1] += np.einsum('kl,lcp->kcp', coef3, M_0l)

    # term 4: anti-diagonal fold of varphi[j,i] phi_ext[k,l] comb[l,i] M_il[i,l]
    comb = sigma_ext[:, None] * lam[None, :]            # (H+1, H) -> [l, i]
    corr = np.zeros((2 * M - 1, H + 1, H), f8)          # [d, l, i]
    for j in range(M):
        for k in range(M):
            corr[j + k] += phi_ext[k][:, None] * varphi[j][None, :]
    C4 = corr * comb[None]                              # (2M-1, H+1, H)
    W[1:2 * M] += np.einsum('dli,ilcp->dcp', C4, M_il)

    # reorder to ascending time index: W_time[j] = W_lag[63 - j]
    # and lay out for SBUF: w_dev[p, j, c]
    w_dev = np.ascontiguousarray(W[::-1].transpose(2, 0, 1)).astype(np.float32)
    return w_dev


def _transpose_shards(y, npdt):
    """y (B, L, P) fp32 -> list of per-core yt (P, NLAG, BS) npdt,
    yt[p, j, b] = y[core*BS + b, j + 2, p]."""
    src = y[:, 2:, :]                  # (B, 64, 128) strided view
    shards = [np.empty((P, NLAG, BS), npdt) for _ in range(N_CORES)]
    BB = 128

    def work(args):
        ci, b0 = args
        blk = np.ascontiguousarray(src[ci * BS + b0: ci * BS + b0 + BB])
        if npdt != np.float32:
            blk = blk.astype(npdt)
        shards[ci][:, :, b0:b0 + BB] = blk.transpose(2, 1, 0)

    jobs = [(ci, b0) for ci in range(N_CORES) for b0 in range(0, BS, BB)]
    with ThreadPoolExecutor(8) as ex:
        list(ex.map(work, jobs))
    return shards


def _mode_params(mode):
    from concourse import mybir
    f32 = dict(np=np.float32, my=mybir.dt.float32)
    f16 = dict(np=np.float16, my=mybir.dt.float16)
    e3 = dict(np=ml_dtypes.float8_e3m4, my=mybir.dt.float8e3)
    if mode == "f8":
        return dict(y=e3, w=f16, o=f16, rdt=None, kj=4, ybufs=16)
    if mode == "f16":
        return dict(y=f16, w=f16, o=f32, rdt=None, kj=4, ybufs=8)
    if mode == "f32r":
        return dict(y=f32, w=f32, o=f32, rdt="float32r", kj=2, ybufs=8)
    if mode == "f32":
        return dict(y=f32, w=f32, o=f32, rdt=None, kj=2, ybufs=8)
    raise ValueError(mode)


def _build_nc(mode):
    import concourse.bass as bass
    import concourse.tile as tile
    from concourse import mybir
    from concourse.bass import ts
    from contextlib import ExitStack

    mp = _mode_params(mode)
    YDT = mp["y"]["my"]
    WDT = mp["w"]["my"]
    ODT = mp["o"]["my"]
    RDT = getattr(mybir.dt, mp["rdt"]) if mp["rdt"] else None
    YMM = RDT or YDT     # dtype seen by the matmul / SBUF tiles
    WMM = RDT or WDT
    KJ = mp["kj"]

    nc = bass.Bass()
    yt = nc.declare_dram_parameter("yt", [P, NLAG, BS], YDT, isOutput=False)
    w = nc.declare_dram_parameter("w", [P, NLAG, MC], WDT, isOutput=False)
    out = nc.declare_dram_parameter("out", [MC, BS], ODT, isOutput=True)

    NSUP = NLAG // KJ  # DMA super-chunks

    with ExitStack() as ctx:
        tc = ctx.enter_context(tile.TileContext(nc))
        wpool = ctx.enter_context(tc.tile_pool(name="wp", bufs=1))
        ypool = ctx.enter_context(
            tc.tile_pool(name="yp", bufs=mp["ybufs"]))
        opool = ctx.enter_context(tc.tile_pool(name="op", bufs=1))
        rpool = ctx.enter_context(tc.tile_pool(name="rp", bufs=1))
        pspool = ctx.enter_context(tc.tile_pool(name="ps", bufs=1, space="PSUM"))

        wtile = wpool.tile([P, NLAG, MC], WMM)
        wsrc = w[:]
        if RDT is not None:
            wsrc = wsrc.bitcast(RDT)
        nc.sync.dma_start(wtile[:], wsrc)

        psums = [pspool.tile([MC, 512], mybir.dt.float32, name=f"ps{t}")
                 for t in range(NBT)]

        # Warm-up: ramp the HAM clock gate toward 8/8 while the w/y DMAs
        # stream.  The ramp tile is memset locally so the PE can start at
        # t~=0; the final warm matmul consumes wtile, giving the PE a
        # single-wait observation of the w-DMA (walrus rejects >1 sync
        # wait on a matmul, so the first real matmul may only wait on y).
        warm = pspool.tile([MC, 512], mybir.dt.float32, name="warm")
        ramp = rpool.tile([P, 512], WMM)
        nc.vector.memset(ramp[:], 0.5)
        NW = 10
        for wi in range(NW):
            nc.tensor.matmul(warm[:], ramp[:, 0:128], ramp[:],
                             start=(wi == 0), stop=False)
        nc.tensor.matmul(warm[:], wtile[:, 0, :], ramp[:],
                         start=False, stop=True)

        for kk in range(NSUP):
            ytile = ypool.tile([P, KJ, BS], YMM)
            src = yt[:, kk * KJ:(kk + 1) * KJ, :]
            if RDT is not None:
                src = src.bitcast(RDT)
            nc.sync.dma_start(ytile[:], src)
            for jj in range(KJ):
                j = kk * KJ + jj
                lhsT = wtile[:, j, :]
                for t in range(NBT):
                    rhs = ytile[:, jj, ts(t, 512)]
                    nc.tensor.matmul(psums[t][:], lhsT, rhs,
                                     start=(j == 0), stop=(j == NLAG - 1))

        outt = opool.tile([MC, BS], ODT)
        for t in range(NBT):
            nc.vector.tensor_copy(outt[:, ts(t, 512)], psums[t][:])
        nc.sync.dma_start(out[:], outt[:])

    return nc


def _strip_redundant_waits(nc):
    """Drop semaphore waits that are provably implied by other waits.

    Tile's add_semaphores pass is per-processor minimal but not transitively
    minimal; walrus codegen allows only one sync wait per DMA/Matmult/Drain
    HW instruction.  Model:
      - expand(s >= v) = {s >= v} union C[sat(s, v)] where sat is the
        instruction whose semaphore update first reaches v (updates on one
        engine sem / one DMA lane fire in order).
      - C[i] ("true once i's updates fired") = own updates + expand(own
        waits) + dispatch-knowledge (expand of same-engine predecessors'
        waits; sequencers evaluate waits in program order) + C[predecessor]
        chained in completion order: same engine for compute engines, same
        DMA lane for DMACopy (async transfers complete in ring order).
    A wait w on i is droppable iff w is in (expand of same-engine
    predecessors' waits) union (expand of i's other waits).
    """
    from concourse import mybir

    f = nc.m.functions[0]
    insts = [i for blk in f.blocks for i in blk.instructions]

    def waits(i):
        si = i.sync_info
        return [(w.ant_name, w.wait_value) for w in (si.on_wait or [])] \
            if si else []

    def updates(i):
        si = i.sync_info
        return list(si.on_update or []) if si else []

    by_engine = {}
    for i in insts:
        by_engine.setdefault(str(i.engine), []).append(i)

    COMPUTE = {"EngineType.PE", "EngineType.DVE", "EngineType.Activation",
               "EngineType.Pool"}

    # cumulative update values per sem, in program order of the updater
    sem_updates = {}           # sem -> [(inst_name, cumulative)]
    upd_of = {i.name: [] for i in insts}
    for eng, lst in by_engine.items():
        for i in lst:
            for u in updates(i):
                cum = sem_updates.setdefault(u.ant_name, [])
                prev = cum[-1][1] if cum else 0
                val = u.update_value if u.update_mode == "sem-add-imm" else 1
                cum.append((i.name, prev + val))
                upd_of[i.name].append((u.ant_name, prev + val))

    def satisfier(sem, v):
        for name, val in sem_updates.get(sem, ()):
            if val >= v:
                return name
        return None

    # completion-order predecessor: same engine (compute) or same DMA lane
    comp_pred = {}
    last_on_lane = {}
    for eng, lst in by_engine.items():
        prev = None
        for i in lst:
            if type(i).__name__ == "InstDMACopy":
                lanes = [s for s, _ in upd_of[i.name]]
                lane = lanes[0] if lanes else None
                comp_pred[i.name] = last_on_lane.get(lane)
                if lane is not None:
                    last_on_lane[lane] = i.name
            elif eng in COMPUTE:
                comp_pred[i.name] = prev
            else:
                comp_pred[i.name] = None
            prev = i.name

    # dispatch-order predecessor (same engine, any type)
    disp_pred = {}
    for eng, lst in by_engine.items():
        prev = None
        for i in lst:
            disp_pred[i.name] = prev
            prev = i.name

    C = {i.name: {} for i in insts}
    DW = {i.name: {} for i in insts}   # dispatch knowledge (pred waits, cum.)
    name2inst = {i.name: i for i in insts}

    def merge(dst, src_items):
        ch = False
        for s, v in src_items:
            if dst.get(s, 0) < v:
                dst[s] = v
                ch = True
        return ch

    changed = True
    rounds = 0
    while changed and rounds < 100:
        changed = False
        rounds += 1
        for i in insts:
            n = i.name
            # DW: dispatch knowledge = pred's DW + expand(pred's waits)
            dp = disp_pred[n]
            if dp is not None:
                changed |= merge(DW[n], DW[dp].items())
                for s, v in waits(name2inst[dp]):
                    changed |= merge(DW[n], [(s, v)])
                    j = satisfier(s, v)
                    if j is not None:
                        changed |= merge(DW[n], C[j].items())
            # C: completion closure
            changed |= merge(C[n], DW[n].items())
            changed |= merge(C[n], upd_of[n])
            cp = comp_pred.get(n)
            if cp is not None:
                changed |= merge(C[n], C[cp].items())
            for s, v in waits(i):
                changed |= merge(C[n], [(s, v)])
                j = satisfier(s, v)
                if j is not None:
                    changed |= merge(C[n], C[j].items())

    for i in insts:
        si = i.sync_info
        if not si or len(si.on_wait or []) <= 1:
            continue
        if type(i).__name__ not in ("InstDMACopy", "InstMatmult",
                                    "InstDrain"):
            continue
        keep = []
        for w in si.on_wait:
            avail = dict(DW[i.name])
            for w2 in si.on_wait:
                if w2 is w:
                    continue
                merge(avail, [(w2.ant_name, w2.wait_value)])
                j = satisfier(w2.ant_name, w2.wait_value)
                if j is not None:
                    merge(avail, C[j].items())
            if avail.get(w.ant_name, 0) < w.wait_value:
                keep.append(w)
        if len(keep) > 1:
            raise RuntimeError(
                f"{i.name}: still {len(keep)} waits after stripping: "
                f"{[(w.ant_name, w.wait_value) for w in keep]}")
        if len(keep) != len(si.on_wait):
            i.sync_info = mybir.SyncInfo(
                on_wait=keep, on_update=list(si.on_update or []))
    return nc


def _get_nc(mode):
    if mode not in _cached_nc:
        _cached_nc[mode] = _strip_redundant_waits(_build_nc(mode))
    return _cached_nc[mode]


def kernel(y_hist, M_bar_0, M_bar, M_0l, M_il, sigma_powered, phi,
           lambda_powered, varphi):
    from concourse.bass_utils import run_bass_kernel_spmd

    mp = _mode_params(MODE)
    y_hist = np.ascontiguousarray(np.asarray(y_hist, dtype=np.float32))
    w_dev = _fold_weights(np.asarray(M_bar_0), np.asarray(M_bar),
                          np.asarray(M_0l), np.asarray(M_il),
                          np.asarray(sigma_powered), np.asarray(phi),
                          np.asarray(lambda_powered), np.asarray(varphi))
    if mp["w"]["np"] != np.float32:
        w_dev = w_dev.astype(mp["w"]["np"])
    shards = _transpose_shards(y_hist, mp["y"]["np"])

    nc = _get_nc(MODE)
    in_maps = [{"yt": shards[ci], "w": w_dev} for ci in range(N_CORES)]
    res = run_bass_kernel_spmd(nc, in_maps, list(range(N_CORES)), trace=TRACE)

    if TRACE:
        kernel.last_result = res

    out = np.empty((B, MC), np.float32)
    for ci in range(N_CORES):
        out[ci * BS:(ci + 1) * BS] = \
            res.results[ci]["out"].T.astype(np.float32)
    return out


# revision 7
# speedup vs baseline: 1.6537x; 1.0784x over previous
"""Trainium2 Bass kernel for the spectral history-filter model (nn_DSC_23252952941334).

Math: all four reference terms are linear in y_hist with per-lag effective
weight matrices, so the whole module collapses to

    out[b, c] = sum_{j=0..63} sum_p  W_time[j][c, p] * y_hist[b, j+2, p]

where W_time[j] combines M_bar_0 / M_bar / M_0l / M_il with the small
spectral coefficient vectors (folded on host in float64 — ~5 MFLOP).

Device strategy (8 cores, data-parallel over batch), "f8" mode:
  - precision: y in float8_e3m4 (halves the HBM traffic that bound the f16
    version), w in float16 stationary.  Mixed-dtype matmul verified exact
    on HW (FP22 datapath, e3m4 subnormals OK).  End-to-end rel err 1.15e-2
    against the 2e-2 gate (f16 was 2.5e-4).
  - layout: ONE packed DRAM tensor per core, wy[p, j, 0:128]=w_j f16 and
    wy[p, j, 128:1152]=y lag j as fp8 pairs; each chunked DMA delivers the
    weights together with the y data it needs, so every matmul needs only
    a single DMA-sem wait (walrus allows one sync wait per HW instruction).
  - schedule: small leading chunks (1,1,2 lags) so the PE starts ~10us
    after kernel start; a dependency-free warm-matmul chain ramps the HAM
    clock gate (PE starts throttled 4/8 by default) while DMA pipes up.
  - compute: 64 accumulated 128-contraction matmuls into 4 PSUM banks
    (512-wide b tiles); steady state measured at the 216 ns/matmul roofline.
  - drain: last chunk runs bank-major so PSUM banks retire early; DVE and
    ACT evacuate two banks each in parallel into f16, two output DMAs.
"""

import os
import numpy as np
import ml_dtypes
from concurrent.futures import ThreadPoolExecutor

N_CORES = 8
B, L, P, MC = 16384, 66, 128, 128
H, M = 16, 32
NLAG = 64            # lags d=0..63 <-> y time indices 2..65
BS = B // N_CORES    # 2048 batch rows per core
NBT = 4              # psum b-tiles of 512
WCOL = MC            # f16 columns of w per lag in the packed tensor
YCOL = BS // 2       # f16 columns holding the fp8 y pairs
PCOL = WCOL + YCOL   # 1152
CHUNKS = [1, 1, 2] + [4] * 15   # lags per DMA chunk (sum = 64)
NWARM = 30           # HAM-ramp matmuls before real work

MODE = os.environ.get("KERNEL_MODE", "f8")
TRACE = False        # test.py can flip this to get a profile

_cached_nc = {}


def _fold_weights(M_bar_0, M_bar, M_0l, M_il, sigma_powered, phi,
                  lambda_powered, varphi):
    """Return w_dev (P, NLAG, MC) fp32 with w_dev[p, j, c] = W_time[j][c, p]."""
    f8 = np.float64
    M_bar_0 = M_bar_0.astype(f8); M_bar = M_bar.astype(f8)
    M_0l = M_0l.astype(f8); M_il = M_il.astype(f8)
    sig = sigma_powered.astype(f8); lam = lambda_powered.astype(f8)
    phi = phi.astype(f8); varphi = varphi.astype(f8)

    # W_lag[d] acts on Yr[:, d] = y[:, L-1-d]
    W = np.zeros((NLAG, MC, P), f8)
    W[0] = M_bar_0

    # term 2: sum_i lam[i] * varphi[j, i] * M_bar[i] on lag j+1
    coef2 = varphi @ np.diag(lam)                       # (M, H) -> [j, i]
    W[1:M + 1] += np.einsum('ji,icp->jcp', coef2, M_bar)

    # term 3: sum_l sigma_ext[l] * phi_ext[k, l] * M_0l[l] on lag k+1
    sigma_ext = np.concatenate([[1.0], sig])            # (H+1,)
    phi_ext = np.concatenate([np.ones((M, 1)), phi], 1)  # (M, H+1)
    coef3 = phi_ext @ np.diag(sigma_ext)                # (M, H+1) -> [k, l]
    W[1:M + 1] += np.einsum('kl,lcp->kcp', coef3, M_0l)

    # term 4: anti-diagonal fold of varphi[j,i] phi_ext[k,l] comb[l,i] M_il[i,l]
    comb = sigma_ext[:, None] * lam[None, :]            # (H+1, H) -> [l, i]
    corr = np.zeros((2 * M - 1, H + 1, H), f8)          # [d, l, i]
    for j in range(M):
        for k in range(M):
            corr[j + k] += phi_ext[k][:, None] * varphi[j][None, :]
    C4 = corr * comb[None]                              # (2M-1, H+1, H)
    W[1:2 * M] += np.einsum('dli,ilcp->dcp', C4, M_il)

    # reorder to ascending time index: W_time[j] = W_lag[63 - j]
    # and lay out for SBUF: w_dev[p, j, c]
    w_dev = np.ascontiguousarray(W[::-1].transpose(2, 0, 1)).astype(np.float32)
    return w_dev


def _pack_shards(y, w_dev):
    """Pack per-core wy (P, NLAG, PCOL) f16:
    [:, j, :WCOL] = w_dev[:, j, :] f16,
    [:, j, WCOL:] = y[core*BS + b, j + 2, p] as e3m4 byte pairs."""
    src = y[:, 2:, :]                  # (B, 64, 128) strided view
    wb = w_dev.astype(np.float16)      # (P, NLAG, MC)
    shards = []
    for ci in range(N_CORES):
        arr = np.empty((P, NLAG, PCOL), np.float16)
        arr[:, :, :WCOL] = wb
        shards.append(arr)
    BB = 128

    def work(args):
        ci, b0 = args
        blk = np.ascontiguousarray(src[ci * BS + b0: ci * BS + b0 + BB])
        q = blk.astype(ml_dtypes.float8_e3m4).view(np.uint8)  # (BB, 64, 128)
        u8 = shards[ci].view(np.uint8)  # (P, NLAG, 2*PCOL), y bytes at 2*WCOL+
        u8[:, :, 2 * WCOL + b0: 2 * WCOL + b0 + BB] = q.transpose(2, 1, 0)

    jobs = [(ci, b0) for ci in range(N_CORES) for b0 in range(0, BS, BB)]
    with ThreadPoolExecutor(8) as ex:
        list(ex.map(work, jobs))
    return shards


def _build_nc_f8():
    import concourse.bass as bass
    import concourse.tile as tile
    from concourse import mybir
    from concourse.bass import ts
    from contextlib import ExitStack

    F16 = mybir.dt.float16
    E3 = mybir.dt.float8e3

    nc = bass.Bass()
    wy = nc.declare_dram_parameter("wy", [P, NLAG, PCOL], F16, isOutput=False)
    out = nc.declare_dram_parameter("out", [MC, BS], F16, isOutput=True)

    spans = []
    s = 0
    for cl in CHUNKS:
        spans.append((s, cl))
        s += cl
    assert s == NLAG

    with ExitStack() as ctx:
        tc = ctx.enter_context(tile.TileContext(nc))
        ypool = ctx.enter_context(tc.tile_pool(name="yp", bufs=len(CHUNKS)))
        opool = ctx.enter_context(tc.tile_pool(name="op", bufs=1))
        rpool = ctx.enter_context(tc.tile_pool(name="rp", bufs=1))
        pspool = ctx.enter_context(tc.tile_pool(name="ps", bufs=1, space="PSUM"))

        psums = [pspool.tile([MC, 512], mybir.dt.float32, name=f"ps{t}")
                 for t in range(NBT)]

        # HAM-ramp chain: the PE clock gate defaults to 4/8 duty and needs
        # ~3.4us of sustained activity to open.  Run small matmuls on a
        # memset tile while the DMA pipe spins up (~10us) so real matmuls
        # start at full rate.
        warm = pspool.tile([MC, 512], mybir.dt.float32, name="warm")
        ramp = rpool.tile([P, 128], F16)
        nc.vector.memset(ramp[:], 0.5)
        for wi in range(NWARM):
            nc.tensor.matmul(warm[:, 0:128], ramp[:], ramp[:],
                             start=(wi == 0), stop=(wi == NWARM - 1))

        ytiles = []
        for ci, (s, cl) in enumerate(spans):
            ytile = ypool.tile([P, cl, PCOL], F16)
            nc.sync.dma_start(ytile[:], wy[:, s:s + cl, :])
            ytiles.append(ytile)
            last = ci == len(spans) - 1
            # bank-major on the last chunk so psum banks retire early
            order = [(t, jj) for t in range(NBT) for jj in range(cl)] \
                if last else [(t, jj) for jj in range(cl) for t in range(NBT)]
            for t, jj in order:
                j = s + jj
                lhsT = ytile[:, jj, 0:WCOL]
                rhs = ytile[:, jj,
                            WCOL + 256 * t: WCOL + 256 * (t + 1)].bitcast(E3)
                nc.tensor.matmul(psums[t][:], lhsT, rhs,
                                 start=(j == 0), stop=(j == NLAG - 1))

        # parallel evacuation: DVE takes banks 0-2 as they retire, ACT the
        # last bank.  A 1-column DVE "observer" of ACT's output funnels the
        # completion into the DVE sem so the single out-DMA (and the final
        # drain) need only one sync wait each (walrus limit).
        outt = opool.tile([MC, BS], F16)
        nc.vector.tensor_copy(outt[:, ts(0, 512)], psums[0][:])
        nc.vector.tensor_copy(outt[:, ts(1, 512)], psums[1][:])
        nc.scalar.copy(outt[:, ts(3, 512)], psums[3][:])
        nc.vector.tensor_copy(outt[:, ts(2, 512)], psums[2][:])
        nc.vector.tensor_scalar_add(outt[:, 2047:2048], outt[:, 2047:2048],
                                    0.0)
        nc.sync.dma_start(out[:], outt[:])

    return nc


def _strip_redundant_waits(nc):
    """Drop semaphore waits that are provably implied by other waits.

    Tile's add_semaphores pass is per-processor minimal but not transitively
    minimal; walrus codegen allows only one sync wait per DMA/Matmult/Drain
    HW instruction.  Model:
      - expand(s >= v) = {s >= v} union C[sat(s, v)] where sat is the
        instruction whose semaphore update first reaches v (updates on one
        engine sem / one DMA lane fire in order).
      - C[i] ("true once i's updates fired") = own updates + expand(own
        waits) + dispatch-knowledge (expand of same-engine predecessors'
        waits; sequencers evaluate waits in program order) + C[predecessor]
        chained in completion order: same engine for compute engines, same
        DMA lane for DMACopy (async transfers complete in ring order).
    A wait w on i is droppable iff w is in (expand of same-engine
    predecessors' waits) union (expand of i's other waits).
    """
    from concourse import mybir

    f = nc.m.functions[0]
    insts = [i for blk in f.blocks for i in blk.instructions]

    def waits(i):
        si = i.sync_info
        return [(w.ant_name, w.wait_value) for w in (si.on_wait or [])] \
            if si else []

    def updates(i):
        si = i.sync_info
        return list(si.on_update or []) if si else []

    by_engine = {}
    for i in insts:
        by_engine.setdefault(str(i.engine), []).append(i)

    COMPUTE = {"EngineType.PE", "EngineType.DVE", "EngineType.Activation",
               "EngineType.Pool"}

    # cumulative update values per sem, in program order of the updater
    sem_updates = {}           # sem -> [(inst_name, cumulative)]
    upd_of = {i.name: [] for i in insts}
    for eng, lst in by_engine.items():
        for i in lst:
            for u in updates(i):
                cum = sem_updates.setdefault(u.ant_name, [])
                prev = cum[-1][1] if cum else 0
                val = u.update_value if u.update_mode == "sem-add-imm" else 1
                cum.append((i.name, prev + val))
                upd_of[i.name].append((u.ant_name, prev + val))

    def satisfier(sem, v):
        for name, val in sem_updates.get(sem, ()):
            if val >= v:
                return name
        return None

    # completion-order predecessor: same engine (compute) or same DMA lane
    comp_pred = {}
    last_on_lane = {}
    for eng, lst in by_engine.items():
        prev = None
        for i in lst:
            if type(i).__name__ == "InstDMACopy":
                lanes = [s for s, _ in upd_of[i.name]]
                lane = lanes[0] if lanes else None
                comp_pred[i.name] = last_on_lane.get(lane)
                if lane is not None:
                    last_on_lane[lane] = i.name
            elif eng in COMPUTE:
                comp_pred[i.name] = prev
            else:
                comp_pred[i.name] = None
            prev = i.name

    # dispatch-order predecessor (same engine, any type)
    disp_pred = {}
    for eng, lst in by_engine.items():
        prev = None
        for i in lst:
            disp_pred[i.name] = prev
            prev = i.name

    C = {i.name: {} for i in insts}
    DW = {i.name: {} for i in insts}   # dispatch knowledge (pred waits, cum.)
    name2inst = {i.name: i for i in insts}

    def merge(dst, src_items):
        ch = False
        for s, v in src_items:
            if dst.get(s, 0) < v:
                dst[s] = v
                ch = True
        return ch

    changed = True
    rounds = 0
    while changed and rounds < 100:
        changed = False
        rounds += 1
        for i in insts:
            n = i.name
            # DW: dispatch knowledge = pred's DW + expand(pred's waits)
            dp = disp_pred[n]
            if dp is not None:
                changed |= merge(DW[n], DW[dp].items())
                for s, v in waits(name2inst[dp]):
                    changed |= merge(DW[n], [(s, v)])
                    j = satisfier(s, v)
                    if j is not None:
                        changed |= merge(DW[n], C[j].items())
            # C: completion closure
            changed |= merge(C[n], DW[n].items())
            changed |= merge(C[n], upd_of[n])
            cp = comp_pred.get(n)
            if cp is not None:
                changed |= merge(C[n], C[cp].items())
            for s, v in waits(i):
                changed |= merge(C[n], [(s, v)])
                j = satisfier(s, v)
                if j is not None:
                    changed |= merge(C[n], C[j].items())

    for i in insts:
        si = i.sync_info
        if not si or len(si.on_wait or []) <= 1:
            continue
        if type(i).__name__ not in ("InstDMACopy", "InstMatmult",
                                    "InstDrain"):
            continue
        keep = []
        for w in si.on_wait:
            avail = dict(DW[i.name])
            for w2 in si.on_wait:
                if w2 is w:
                    continue
                merge(avail, [(w2.ant_name, w2.wait_value)])
                j = satisfier(w2.ant_name, w2.wait_value)
                if j is not None:
                    merge(avail, C[j].items())
            if avail.get(w.ant_name, 0) < w.wait_value:
                keep.append(w)
        if len(keep) > 1:
            raise RuntimeError(
                f"{i.name}: still {len(keep)} waits after stripping: "
                f"{[(w.ant_name, w.wait_value) for w in keep]}")
        if len(keep) != len(si.on_wait):
            i.sync_info = mybir.SyncInfo(
                on_wait=keep, on_update=list(si.on_update or []))
    return nc


def _get_nc():
    if "f8" not in _cached_nc:
        _cached_nc["f8"] = _strip_redundant_waits(_build_nc_f8())
    return _cached_nc["f8"]


def kernel(y_hist, M_bar_0, M_bar, M_0l, M_il, sigma_powered, phi,
           lambda_powered, varphi):
    from concourse.bass_utils import run_bass_kernel_spmd

    y_hist = np.ascontiguousarray(np.asarray(y_hist, dtype=np.float32))
    w_dev = _fold_weights(np.asarray(M_bar_0), np.asarray(M_bar),
                          np.asarray(M_0l), np.asarray(M_il),
                          np.asarray(sigma_powered), np.asarray(phi),
                          np.asarray(lambda_powered), np.asarray(varphi))
    shards = _pack_shards(y_hist, w_dev)

    nc = _get_nc()
    in_maps = [{"wy": shards[ci]} for ci in range(N_CORES)]
    res = run_bass_kernel_spmd(nc, in_maps, list(range(N_CORES)), trace=TRACE)

    if TRACE:
        kernel.last_result = res

    out = np.empty((B, MC), np.float32)
    for ci in range(N_CORES):
        out[ci * BS:(ci + 1) * BS] = \
            res.results[ci]["out"].T.astype(np.float32)
    return out


# revision 10
# speedup vs baseline: 1.6661x; 1.0075x over previous
"""Trainium2 Bass kernel for the spectral history-filter model (nn_DSC_23252952941334).

Math: all four reference terms are linear in y_hist with per-lag effective
weight matrices, so the whole module collapses to

    out[b, c] = sum_{j=0..63} sum_p  W_time[j][c, p] * y_hist[b, j+2, p]

where W_time[j] combines M_bar_0 / M_bar / M_0l / M_il with the small
spectral coefficient vectors (folded on host in float64 — ~5 MFLOP).

Device strategy (8 cores, data-parallel over batch), "f8" mode:
  - precision: y in float8_e3m4 (halves the HBM traffic that bound the f16
    version), w in float16 stationary.  Mixed-dtype matmul verified exact
    on HW (FP22 datapath, e3m4 subnormals OK).  End-to-end rel err 1.15e-2
    against the 2e-2 gate (f16 was 2.5e-4).
  - layout: ONE packed DRAM tensor per core, wy[p, j, 0:128]=w_j f16 and
    wy[p, j, 128:1152]=y lag j as fp8 pairs; each chunked DMA delivers the
    weights together with the y data it needs, so every matmul needs only
    a single DMA-sem wait (walrus allows one sync wait per HW instruction).
  - schedule: small leading chunks (1,1,2 lags) so the PE starts ~10us
    after kernel start; a dependency-free warm-matmul chain ramps the HAM
    clock gate (PE starts throttled 4/8 by default) while DMA pipes up.
  - compute: 64 accumulated 128-contraction matmuls into 4 PSUM banks
    (512-wide b tiles); steady state measured at the 216 ns/matmul roofline.
  - drain: last chunk runs bank-major so PSUM banks retire early; DVE and
    ACT evacuate two banks each in parallel into f16, two output DMAs.
"""

import os
import numpy as np
import ml_dtypes
from concurrent.futures import ThreadPoolExecutor

N_CORES = 8
B, L, P, MC = 16384, 66, 128, 128
H, M = 16, 32
NLAG = 64            # lags d=0..63 <-> y time indices 2..65
BS = B // N_CORES    # 2048 batch rows per core
NBT = 4              # psum b-tiles of 512
WCOL = MC            # f16 columns of w per lag in the packed tensor
YCOL = BS // 2       # f16 columns holding the fp8 y pairs
PCOL = WCOL + YCOL   # 1152
CHUNKS = [1] * 8 + [2] * 8 + [4] * 10   # lags per DMA chunk (sum = 64)
NWARM = 13           # HAM-ramp matmuls before real work

MODE = os.environ.get("KERNEL_MODE", "f8")
TRACE = False        # test.py can flip this to get a profile

_cached_nc = {}


def _fold_weights(M_bar_0, M_bar, M_0l, M_il, sigma_powered, phi,
                  lambda_powered, varphi):
    """Return w_dev (P, NLAG, MC) fp32 with w_dev[p, j, c] = W_time[j][c, p]."""
    f8 = np.float64
    M_bar_0 = M_bar_0.astype(f8); M_bar = M_bar.astype(f8)
    M_0l = M_0l.astype(f8); M_il = M_il.astype(f8)
    sig = sigma_powered.astype(f8); lam = lambda_powered.astype(f8)
    phi = phi.astype(f8); varphi = varphi.astype(f8)

    # W_lag[d] acts on Yr[:, d] = y[:, L-1-d]
    W = np.zeros((NLAG, MC, P), f8)
    W[0] = M_bar_0

    # term 2: sum_i lam[i] * varphi[j, i] * M_bar[i] on lag j+1
    coef2 = varphi @ np.diag(lam)                       # (M, H) -> [j, i]
    W[1:M + 1] += np.einsum('ji,icp->jcp', coef2, M_bar)

    # term 3: sum_l sigma_ext[l] * phi_ext[k, l] * M_0l[l] on lag k+1
    sigma_ext = np.concatenate([[1.0], sig])            # (H+1,)
    phi_ext = np.concatenate([np.ones((M, 1)), phi], 1)  # (M, H+1)
    coef3 = phi_ext @ np.diag(sigma_ext)                # (M, H+1) -> [k, l]
    W[1:M + 1] += np.einsum('kl,lcp->kcp', coef3, M_0l)

    # term 4: anti-diagonal fold of varphi[j,i] phi_ext[k,l] comb[l,i] M_il[i,l]
    comb = sigma_ext[:, None] * lam[None, :]            # (H+1, H) -> [l, i]
    corr = np.zeros((2 * M - 1, H + 1, H), f8)          # [d, l, i]
    for j in range(M):
        for k in range(M):
            corr[j + k] += phi_ext[k][:, None] * varphi[j][None, :]
    C4 = corr * comb[None]                              # (2M-1, H+1, H)
    W[1:2 * M] += np.einsum('dli,ilcp->dcp', C4, M_il)

    # reorder to ascending time index: W_time[j] = W_lag[63 - j]
    # and lay out for SBUF: w_dev[p, j, c]
    w_dev = np.ascontiguousarray(W[::-1].transpose(2, 0, 1)).astype(np.float32)
    return w_dev


def _pack_shards(y, w_dev):
    """Pack per-core wy (P, NLAG, PCOL) f16:
    [:, j, :WCOL] = w_dev[:, j, :] f16,
    [:, j, WCOL:] = y[core*BS + b, j + 2, p] as e3m4 byte pairs."""
    src = y[:, 2:, :]                  # (B, 64, 128) strided view
    wb = w_dev.astype(np.float16)      # (P, NLAG, MC)
    shards = []
    for ci in range(N_CORES):
        arr = np.empty((P, NLAG, PCOL), np.float16)
        arr[:, :, :WCOL] = wb
        shards.append(arr)
    BB = 128

    def work(args):
        ci, b0 = args
        blk = np.ascontiguousarray(src[ci * BS + b0: ci * BS + b0 + BB])
        q = blk.astype(ml_dtypes.float8_e3m4).view(np.uint8)  # (BB, 64, 128)
        u8 = shards[ci].view(np.uint8)  # (P, NLAG, 2*PCOL), y bytes at 2*WCOL+
        u8[:, :, 2 * WCOL + b0: 2 * WCOL + b0 + BB] = q.transpose(2, 1, 0)

    jobs = [(ci, b0) for ci in range(N_CORES) for b0 in range(0, BS, BB)]
    with ThreadPoolExecutor(8) as ex:
        list(ex.map(work, jobs))
    return shards


def _build_nc_f8():
    import concourse.bass as bass
    import concourse.tile as tile
    from concourse import mybir
    from concourse.bass import ts
    from contextlib import ExitStack

    F16 = mybir.dt.float16
    E3 = mybir.dt.float8e3

    nc = bass.Bass()
    wy = nc.declare_dram_parameter("wy", [P, NLAG, PCOL], F16, isOutput=False)
    out = nc.declare_dram_parameter("out", [MC, BS], F16, isOutput=True)

    spans = []
    s = 0
    for cl in CHUNKS:
        spans.append((s, cl))
        s += cl
    assert s == NLAG

    with ExitStack() as ctx:
        tc = ctx.enter_context(tile.TileContext(nc))
        # one pool per chunk size class so ring slots match tile sizes
        pools = {}
        for cl in sorted(set(CHUNKS)):
            n = CHUNKS.count(cl)
            pools[cl] = ctx.enter_context(
                tc.tile_pool(name=f"yp{cl}", bufs=n))
        opool = ctx.enter_context(tc.tile_pool(name="op", bufs=1))
        rpool = ctx.enter_context(tc.tile_pool(name="rp", bufs=1))
        pspool = ctx.enter_context(tc.tile_pool(name="ps", bufs=1, space="PSUM"))

        psums = [pspool.tile([MC, 512], mybir.dt.float32, name=f"ps{t}")
                 for t in range(NBT)]

        # HAM-ramp chain: the PE clock gate defaults to 4/8 duty and needs
        # ~3.4us of sustained activity to open.  Run small matmuls while the
        # DMA pipe spins up (~10us) so real matmuls start at full rate.
        warm = pspool.tile([MC, 512], mybir.dt.float32, name="warm")
        ramp = rpool.tile([P, 128], F16)
        nc.vector.memset(ramp[:], 0.5)
        for wi in range(NWARM):
            nc.tensor.matmul(warm[:, 0:128], ramp[:], ramp[:],
                             start=(wi == 0), stop=(wi == NWARM - 1))

        ytiles = []
        for ci, (s, cl) in enumerate(spans):
            ytile = pools[cl].tile([P, cl, PCOL], F16)
            nc.sync.dma_start(ytile[:], wy[:, s:s + cl, :])
            ytiles.append(ytile)
            last = ci == len(spans) - 1
            # bank-major on the last chunk so psum banks retire early
            order = [(t, jj) for t in range(NBT) for jj in range(cl)] \
                if last else [(t, jj) for jj in range(cl) for t in range(NBT)]
            for t, jj in order:
                j = s + jj
                lhsT = ytile[:, jj, 0:WCOL]
                rhs = ytile[:, jj,
                            WCOL + 256 * t: WCOL + 256 * (t + 1)].bitcast(E3)
                nc.tensor.matmul(psums[t][:], lhsT, rhs,
                                 start=(j == 0), stop=(j == NLAG - 1))

        # evacuation: DVE casts the banks in retirement order (banks 0-2
        # finish 12/8/4 matmuls before bank 3 thanks to the bank-major last
        # chunk), so only bank 3's cast sits on the critical path, and the
        # single out-DMA plus the final drain each need one sync wait.
        outt = opool.tile([MC, BS], F16)
        for t in range(NBT):
            nc.vector.tensor_copy(outt[:, ts(t, 512)], psums[t][:])
        nc.sync.dma_start(out[:], outt[:])

    return nc


def _strip_redundant_waits(nc):
    """Drop semaphore waits that are provably implied by other waits.

    Tile's add_semaphores pass is per-processor minimal but not transitively
    minimal; walrus codegen allows only one sync wait per DMA/Matmult/Drain
    HW instruction.  Model:
      - expand(s >= v) = {s >= v} union C[sat(s, v)] where sat is the
        instruction whose semaphore update first reaches v (updates on one
        engine sem / one DMA lane fire in order).
      - C[i] ("true once i's updates fired") = own updates + expand(own
        waits) + dispatch-knowledge (expand of same-engine predecessors'
        waits; sequencers evaluate waits in program order) + C[predecessor]
        chained in completion order: same engine for compute engines, same
        DMA lane for DMACopy (async transfers complete in ring order).
    A wait w on i is droppable iff w is in (expand of same-engine
    predecessors' waits) union (expand of i's other waits).
    """
    from concourse import mybir

    f = nc.m.functions[0]
    insts = [i for blk in f.blocks for i in blk.instructions]

    def waits(i):
        si = i.sync_info
        return [(w.ant_name, w.wait_value) for w in (si.on_wait or [])] \
            if si else []

    def updates(i):
        si = i.sync_info
        return list(si.on_update or []) if si else []

    by_engine = {}
    for i in insts:
        by_engine.setdefault(str(i.engine), []).append(i)

    COMPUTE = {"EngineType.PE", "EngineType.DVE", "EngineType.Activation",
               "EngineType.Pool"}

    # cumulative update values per sem, in program order of the updater
    sem_updates = {}           # sem -> [(inst_name, cumulative)]
    upd_of = {i.name: [] for i in insts}
    for eng, lst in by_engine.items():
        for i in lst:
            for u in updates(i):
                cum = sem_updates.setdefault(u.ant_name, [])
                prev = cum[-1][1] if cum else 0
                val = u.update_value if u.update_mode == "sem-add-imm" else 1
                cum.append((i.name, prev + val))
                upd_of[i.name].append((u.ant_name, prev + val))

    def satisfier(sem, v):
        for name, val in sem_updates.get(sem, ()):
            if val >= v:
                return name
        return None

    # completion-order predecessor: same engine (compute) or same DMA lane
    comp_pred = {}
    last_on_lane = {}
    for eng, lst in by_engine.items():
        prev = None
        for i in lst:
            if type(i).__name__ == "InstDMACopy":
                lanes = [s for s, _ in upd_of[i.name]]
                lane = lanes[0] if lanes else None
                comp_pred[i.name] = last_on_lane.get(lane)
                if lane is not None:
                    last_on_lane[lane] = i.name
            elif eng in COMPUTE:
                comp_pred[i.name] = prev
            else:
                comp_pred[i.name] = None
            prev = i.name

    # dispatch-order predecessor (same engine, any type)
    disp_pred = {}
    for eng, lst in by_engine.items():
        prev = None
        for i in lst:
            disp_pred[i.name] = prev
            prev = i.name

    C = {i.name: {} for i in insts}
    DW = {i.name: {} for i in insts}   # dispatch knowledge (pred waits, cum.)
    name2inst = {i.name: i for i in insts}

    def merge(dst, src_items):
        ch = False
        for s, v in src_items:
            if dst.get(s, 0) < v:
                dst[s] = v
                ch = True
        return ch

    changed = True
    rounds = 0
    while changed and rounds < 100:
        changed = False
        rounds += 1
        for i in insts:
            n = i.name
            # DW: dispatch knowledge = pred's DW + expand(pred's waits)
            dp = disp_pred[n]
            if dp is not None:
                changed |= merge(DW[n], DW[dp].items())
                for s, v in waits(name2inst[dp]):
                    changed |= merge(DW[n], [(s, v)])
                    j = satisfier(s, v)
                    if j is not None:
                        changed |= merge(DW[n], C[j].items())
            # C: completion closure
            changed |= merge(C[n], DW[n].items())
            changed |= merge(C[n], upd_of[n])
            cp = comp_pred.get(n)
            if cp is not None:
                changed |= merge(C[n], C[cp].items())
            for s, v in waits(i):
                changed |= merge(C[n], [(s, v)])
                j = satisfier(s, v)
                if j is not None:
                    changed |= merge(C[n], C[j].items())

    for i in insts:
        si = i.sync_info
        if not si or len(si.on_wait or []) <= 1:
            continue
        if type(i).__name__ not in ("InstDMACopy", "InstMatmult",
                                    "InstDrain"):
            continue
        keep = []
        for w in si.on_wait:
            avail = dict(DW[i.name])
            for w2 in si.on_wait:
                if w2 is w:
                    continue
                merge(avail, [(w2.ant_name, w2.wait_value)])
                j = satisfier(w2.ant_name, w2.wait_value)
                if j is not None:
                    merge(avail, C[j].items())
            if avail.get(w.ant_name, 0) < w.wait_value:
                keep.append(w)
        if len(keep) > 1:
            raise RuntimeError(
                f"{i.name}: still {len(keep)} waits after stripping: "
                f"{[(w.ant_name, w.wait_value) for w in keep]}")
        if len(keep) != len(si.on_wait):
            i.sync_info = mybir.SyncInfo(
                on_wait=keep, on_update=list(si.on_update or []))
    return nc


def _get_nc():
    if "f8" not in _cached_nc:
        _cached_nc["f8"] = _strip_redundant_waits(_build_nc_f8())
    return _cached_nc["f8"]


def kernel(y_hist, M_bar_0, M_bar, M_0l, M_il, sigma_powered, phi,
           lambda_powered, varphi):
    from concourse.bass_utils import run_bass_kernel_spmd

    y_hist = np.ascontiguousarray(np.asarray(y_hist, dtype=np.float32))
    w_dev = _fold_weights(np.asarray(M_bar_0), np.asarray(M_bar),
                          np.asarray(M_0l), np.asarray(M_il),
                          np.asarray(sigma_powered), np.asarray(phi),
                          np.asarray(lambda_powered), np.asarray(varphi))
    shards = _pack_shards(y_hist, w_dev)

    nc = _get_nc()
    in_maps = [{"wy": shards[ci]} for ci in range(N_CORES)]
    res = run_bass_kernel_spmd(nc, in_maps, list(range(N_CORES)), trace=TRACE)

    if TRACE:
        kernel.last_result = res

    out = np.empty((B, MC), np.float32)
    for ci in range(N_CORES):
        out[ci * BS:(ci + 1) * BS] = \
            res.results[ci]["out"].T.astype(np.float32)
    return out
